# revision 37
# baseline (speedup 1.0000x reference)
"""Trainium2 Bass kernel for nn_MicroStreamBlock (dual-stream block:
quaternion attention branch + Hamilton-mix MLP branch).

Contract: kernel(**inputs) takes the FULL unsharded inputs (as produced by
reference.setup_inputs) and returns the FULL (4, 2048, 2048) float32 output.
Internally the flattened (8192, 2048) token stream is row-sharded across 8
NeuronCores (1024 rows each; a core pair shares one batch).  The per-batch
softmax-over-time partial sums are combined with a tiny pairwise on-device
AllReduce that overlaps with the MLP-branch GEMMs.
"""

import math
import sys

sys.path.insert(0, "/opt/trn_rl_repo")

import ml_dtypes
import numpy as np

import concourse.bass as bass  # noqa: F401
import concourse.mybir as mybir
import concourse.tile as tile
from concourse import bacc
from concourse.bass_utils import run_bass_kernel_spmd

BF16 = ml_dtypes.bfloat16
F32 = mybir.dt.float32
BF = mybir.dt.bfloat16
AF = mybir.ActivationFunctionType
ALU = mybir.AluOpType
AX = mybir.AxisListType

NCORES = 8
B, T, DIM = 4, 2048, 2048
HALF = DIM // 2          # 1024
HEADS, RANK = 4, 8
NQ = (HALF // HEADS) // 4  # 64
GRP = HEADS * NQ           # 256 quaternion groups per token
ROWS = (B * T) // NCORES   # 1024 rows per core
P = 128
KC = HALF // P             # 8 contraction chunks of 128
TC = ROWS // P             # 8 token chunks of 128
LN_EPS = 1e-5
QEPS = 1e-24               # guard for ln of the quat-norm product

_CACHE: dict = {}
_LAST_RESULTS = None


def _build_program(with_bias: bool):
    nc = bacc.Bacc("TRN2", target_bir_lowering=False, debug=False,
                   num_devices=NCORES)

    xc = nc.dram_tensor("xc", [ROWS, DIM], F32, kind="ExternalInput").ap()
    xT = nc.dram_tensor("xT", [DIM, ROWS], BF, kind="ExternalInput").ap()
    wqkv_d = nc.dram_tensor("wqkv", [HALF, 3 * HALF], BF, kind="ExternalInput").ap()
    f1_d = nc.dram_tensor("f1w", [HALF, HALF], BF, kind="ExternalInput").ap()
    f2_d = nc.dram_tensor("f2w", [HALF, HALF], BF, kind="ExternalInput").ap()
    woT_d = nc.dram_tensor("woT", [HALF, HALF], BF, kind="ExternalInput").ap()
    b1_d = nc.dram_tensor("b1e", [HALF, 1], F32, kind="ExternalInput").ap()
    if with_bias:
        bqkv_d = nc.dram_tensor("bqkve", [1, 3 * HALF], BF, kind="ExternalInput").ap()
        b2_d = nc.dram_tensor("b2e", [1, HALF], BF, kind="ExternalInput").ap()
        bo_d = nc.dram_tensor("boe", [1, HALF], BF, kind="ExternalInput").ap()
    out = nc.dram_tensor("out", [ROWS, DIM], F32, kind="ExternalOutput").ap()

    with tile.TileContext(nc) as tc:
        with tc.tile_pool(name="sb", bufs=1) as sb, \
             tc.tile_pool(name="ps", bufs=1, space="PSUM") as ps, \
             tc.tile_pool(name="dp", bufs=1, space="DRAM") as dp:

            # ---------------- constants ----------------
            ones_bf = sb.tile([P, P], BF, tag="ones_bf")
            ones_f = sb.tile([P, P], F32, tag="ones_f")
            nc.vector.memset(ones_bf, 1.0)
            nc.vector.memset(ones_f, 1.0)
            epsln = sb.tile([P, 1], F32, tag="epsln")
            nc.vector.memset(epsln, LN_EPS)
            epsq = sb.tile([P, 1], F32, tag="epsq")
            nc.vector.memset(epsq, QEPS)

            b1cols = sb.tile([P, KC], F32, tag="b1cols")
            for k in range(KC):
                nc.sync.dma_start(out=b1cols[:, k:k + 1],
                                  in_=b1_d[k * P:(k + 1) * P, 0:1])
            if with_bias:
                bqkvr = sb.tile([1, 3 * HALF], BF, tag="bqkvr")
                nc.sync.dma_start(out=bqkvr, in_=bqkv_d)
                b2r = sb.tile([1, HALF], BF, tag="b2r")
                nc.sync.dma_start(out=b2r, in_=b2_d)
                bor = sb.tile([1, HALF], BF, tag="bor")
                nc.sync.dma_start(out=bor, in_=bo_d)

            # ---------------- persistent loads (x2 first: qkv needs it) ----
            x2t = []
            for k in range(KC):
                t2 = sb.tile([P, ROWS], BF, tag="xt", bufs=16, name=f"x2t{k}")
                nc.sync.dma_start(out=t2, in_=xT[HALF + k * P:HALF + (k + 1) * P, :])
                x2t.append(t2)
            x1t = []
            for k in range(KC):
                t1 = sb.tile([P, ROWS], BF, tag="xt", bufs=16, name=f"x1t{k}")
                nc.sync.dma_start(out=t1, in_=xT[k * P:(k + 1) * P, :])
                x1t.append(t1)
            wq_t = []
            for k in range(KC):
                t = sb.tile([P, 3 * HALF], BF, tag="w3", bufs=8, name=f"wq{k}")
                nc.sync.dma_start(out=t, in_=wqkv_d[k * P:(k + 1) * P, :])
                wq_t.append(t)
            f1_t = []
            for k in range(KC):
                t = sb.tile([P, HALF], BF, tag="wf", bufs=8, name=f"f1{k}")
                nc.sync.dma_start(out=t, in_=f1_d[k * P:(k + 1) * P, :])
                f1_t.append(t)
            f2_t = []
            for k in range(KC):
                t = sb.tile([P, HALF], BF, tag="wg", bufs=8, name=f"f2{k}")
                nc.sync.dma_start(out=t, in_=f2_d[k * P:(k + 1) * P, :])
                f2_t.append(t)

            # ---------------- LN stats from bf16 xT (PE column sums) -------
            # istd = exp(-0.5 ln(var+eps)); transient rows rotate 3 slots.
            def ln_stats(xt_tiles, label):
                psx = [ps.tile([1, 512], F32, tag="pB", bufs=4,
                               name=f"psx{label}{h}") for h in range(2)]
                psx2 = [ps.tile([1, 512], F32, tag="pB", bufs=4,
                                name=f"psx2{label}{h}") for h in range(2)]
                for k in range(KC):
                    sq = sb.tile([P, ROWS], BF, tag="sq", bufs=2, name=f"sq{label}{k}")
                    nc.vector.tensor_mul(sq, xt_tiles[k], xt_tiles[k])
                    for h in range(2):
                        nc.tensor.matmul(psx[h], lhsT=ones_bf[:, 0:1],
                                         rhs=xt_tiles[k][:, h * 512:(h + 1) * 512],
                                         start=(k == 0), stop=(k == KC - 1))
                        nc.tensor.matmul(psx2[h], lhsT=ones_bf[:, 0:1],
                                         rhs=sq[:, h * 512:(h + 1) * 512],
                                         start=(k == 0), stop=(k == KC - 1))
                m_row = sb.tile([1, ROWS], F32, tag="rowtmp", bufs=3,
                                name=f"m{label}")
                acc = sb.tile([1, ROWS], F32, tag="rowtmp", bufs=3,
                              name=f"acc{label}")
                for h in range(2):
                    nc.scalar.mul(m_row[0:1, h * 512:(h + 1) * 512], psx[h],
                                  1.0 / HALF)
                    nc.scalar.mul(acc[0:1, h * 512:(h + 1) * 512], psx2[h],
                                  1.0 / HALF)
                # acc: E[x^2] -> var -> ln(var+eps) -> istd   (in place)
                tmp = sb.tile([1, ROWS], F32, tag="rowtmp", bufs=3,
                              name=f"tmp{label}")
                nc.vector.tensor_mul(tmp, m_row, m_row)
                nc.vector.tensor_sub(acc, acc, tmp)
                nc.scalar.activation(acc, acc, AF.Ln, bias=epsln[0:1, 0:1])
                nc.scalar.activation(acc, acc, AF.Exp, scale=-0.5)
                return m_row, acc

            def bcast_row(row, name, half=None):
                """materialize (1,1024) f32 row -> (128,1024) bf16 tile"""
                bt = sb.tile([P, ROWS], BF, tag="bcast", bufs=2, name=name)
                for h in range(2):
                    pb = ps.tile([P, 512], F32, tag="pA", bufs=4,
                                 name=f"pb_{name}{h}")
                    nc.tensor.matmul(pb, lhsT=ones_f[0:1, :],
                                     rhs=row[0:1, h * 512:(h + 1) * 512],
                                     start=True, stop=True)
                    nc.scalar.copy(bt[:, h * 512:(h + 1) * 512], pb)
                return bt

            # stats for both branches first (keeps the PE column-sum matmuls
            # dense); row math overlaps on ACT/DVE
            m2_row, istd2_row = ln_stats(x2t, "b")
            m1_row, istd1_row = ln_stats(x1t, "a")

            # branch f (x2): center in place; istd2 applied at qkv eviction
            m2bc = bcast_row(m2_row, "m2bc")
            for k in range(KC):
                nc.vector.tensor_sub(x2t[k], x2t[k], m2bc)
            istd2c = sb.tile([P, TC], F32, tag="istd2c")
            for c in range(TC):
                pt = ps.tile([P, 1], F32, tag="pA", bufs=4, name=f"ptr{c}")
                nc.tensor.matmul(pt, lhsT=istd2_row[0:1, c * P:(c + 1) * P],
                                 rhs=ones_f[0:1, 0:1], start=True, stop=True)
                nc.scalar.copy(istd2c[:, c:c + 1], pt)
            std2_bf = None
            if with_bias:
                std2_row = sb.tile([1, ROWS], F32, tag="std2row")
                nc.vector.reciprocal(std2_row, istd2_row)
                std2_bf = sb.tile([1, ROWS], BF, tag="std2bf")
                nc.vector.tensor_copy(std2_bf, std2_row)

            # branch g (x1): full normalize in place (istd1 is per-free in
            # the transposed f1 GEMM, so it must be pre-applied)
            m1bc = bcast_row(m1_row, "m1bc")
            istd1bc = bcast_row(istd1_row, "istd1bc")
            for k in range(KC):
                nc.vector.tensor_sub(x1t[k], x1t[k], m1bc)
                nc.vector.tensor_mul(x1t[k], x1t[k], istd1bc)

            # ---------------- stage 1: qkv GEMM + attention partials -------
            nd = [ps.tile([1, 512], F32, tag="pB", bufs=4, name="nd0"),
                  ps.tile([1, 512], F32, tag="pB", bufs=4, name="nd1"),
                  ps.tile([1, 256], F32, tag="pB", bufs=4, name="nd2")]
            nd_slices = [(0, 512), (512, 512), (1024, 256)]

            wds = []
            for c in range(TC):
                q = sb.tile([P, HALF], BF, tag="qk", bufs=3, name=f"q{c}")
                kk_t = sb.tile([P, HALF], BF, tag="qk", bufs=3, name=f"k{c}")
                v = sb.tile([P, HALF], BF, tag="vv", bufs=2, name=f"v{c}")
                dests = [(q, 0), (q, 512), (kk_t, 0), (kk_t, 512), (v, 0), (v, 512)]
                for j in range(6):
                    pm = ps.tile([P, 512], F32, tag="pA", bufs=4, name=f"pqkv{c}_{j}")
                    for k in range(KC):
                        nc.tensor.matmul(pm, lhsT=x2t[k][:, c * P:(c + 1) * P],
                                         rhs=wq_t[k][:, j * 512:(j + 1) * 512],
                                         start=(k == 0),
                                         stop=(not with_bias and k == KC - 1))
                    if with_bias:
                        # psum += outer(std2(t), b(j)); istd2 at eviction
                        # turns this into the plain + b(j).
                        nc.tensor.matmul(pm, lhsT=std2_bf[0:1, c * P:(c + 1) * P],
                                         rhs=bqkvr[0:1, j * 512:(j + 1) * 512],
                                         start=False, stop=True)
                    dt, off = dests[j]
                    nc.scalar.mul(dt[:, off:off + 512], pm, istd2c[:, c:c + 1])

                # quaternion products -> group sums over the 4-vector
                prod = sb.tile([P, HALF], BF, tag="sq", bufs=2, name=f"pr{c}")
                sqq = sb.tile([P, GRP], F32, tag="ss", bufs=3, name=f"sqq{c}")
                skk = sb.tile([P, GRP], F32, tag="ss", bufs=3, name=f"skk{c}")
                sqk = sb.tile([P, GRP], F32, tag="ss", bufs=3, name=f"sqk{c}")
                nc.vector.tensor_mul(prod, q, q)
                nc.vector.tensor_reduce(sqq, prod.rearrange("p (g c) -> p g c", c=4),
                                        axis=AX.X, op=ALU.add)
                nc.vector.tensor_mul(prod, kk_t, kk_t)
                nc.vector.tensor_reduce(skk, prod.rearrange("p (g c) -> p g c", c=4),
                                        axis=AX.X, op=ALU.add)
                nc.vector.tensor_mul(prod, q, kk_t)
                nc.vector.tensor_reduce(sqk, prod.rearrange("p (g c) -> p g c", c=4),
                                        axis=AX.X, op=ALU.add)

                # l = sqk/sqrt(sqq*skk);  e = exp(l/8)   (ln/exp only)
                nc.vector.tensor_mul(sqq, sqq, skk)
                nc.scalar.activation(sqq, sqq, AF.Ln, bias=epsq)
                nc.scalar.activation(sqq, sqq, AF.Exp, scale=-0.5)
                nc.vector.tensor_mul(sqk, sqk, sqq)
                wd = sb.tile([P, HALF + GRP], BF, tag="wd", bufs=3, name=f"wd{c}")
                nc.scalar.activation(wd[:, HALF:], sqk, AF.Exp,
                                     scale=1.0 / math.sqrt(NQ))
                nc.vector.tensor_mul(
                    wd[:, 0:HALF].rearrange("p (g c) -> p g c", c=4),
                    v.rearrange("p (g c) -> p g c", c=4),
                    wd[:, HALF:][:, :, None].to_broadcast([P, GRP, 4]))
                wds.append(wd)
                # numerator/denominator accumulation, deferred one chunk so the
                # attention vector chain stays off the PE critical path
                if c >= 1:
                    for s, (lo, n) in enumerate(nd_slices):
                        nc.tensor.matmul(nd[s], lhsT=ones_bf[:, 0:1],
                                         rhs=wds[c - 1][:, lo:lo + n],
                                         start=(c == 1), stop=False,
                                         skip_group_check=True)
            for s, (lo, n) in enumerate(nd_slices):
                nc.tensor.matmul(nd[s], lhsT=ones_bf[:, 0:1],
                                 rhs=wds[TC - 1][:, lo:lo + n],
                                 start=False, stop=True,
                                 skip_group_check=True)

            # ---------------- pairwise AllReduce of [num | den] -------------
            ndrow = sb.tile([1, HALF + GRP], F32, tag="ndrow")
            for s, (lo, n) in enumerate(nd_slices):
                nc.scalar.copy(ndrow[0:1, lo:lo + n], nd[s])
            ndin = dp.tile([1, HALF + GRP], F32, tag="ndin")
            ndout = dp.tile([1, HALF + GRP], F32, tag="ndout")
            nc.sync.dma_start(out=ndin, in_=ndrow)
            nc.gpsimd.collective_compute(
                "AllReduce", ALU.add,
                replica_groups=[[0, 1], [2, 3], [4, 5], [6, 7]],
                ins=[ndin.opt()], outs=[ndout.opt()])
            ndred = sb.tile([1, HALF + GRP], F32, tag="ndred")
            nc.sync.dma_start(out=ndred, in_=ndout)

            # y1 base copy (DRAM->DRAM; overlaps stage-2 compute) and the
            # out-proj weights (own slots so they load immediately)
            d2d = nc.gpsimd.dma_start(out=out[:, 0:HALF], in_=xc[:, 0:HALF])
            wo_t = []
            for k in range(KC):
                t = sb.tile([P, HALF], BF, tag="wo", bufs=8, name=f"wo{k}")
                nc.scalar.dma_start(out=t, in_=woT_d[k * P:(k + 1) * P, :])
                wo_t.append(t)

            # ---------------- stage 2: Hamilton-mix branch ------------------
            for tt in range(2):
                gts = []
                for jc in range(KC):
                    pm = ps.tile([P, 512], F32, tag="pA", bufs=4,
                                 name=f"pg1_{tt}_{jc}")
                    for k in range(KC):
                        nc.tensor.matmul(pm, lhsT=f1_t[k][:, jc * P:(jc + 1) * P],
                                         rhs=x1t[k][:, tt * 512:(tt + 1) * 512],
                                         start=(k == 0), stop=(k == KC - 1))
                    gt = sb.tile([P, 512], BF, tag="gt", bufs=8, name=f"gt{tt}_{jc}")
                    nc.scalar.activation(gt, pm, AF.Gelu, bias=b1cols[:, jc:jc + 1])
                    gts.append(gt)
                for t2 in range(4):
                    tcg = tt * 4 + t2
                    xn2 = sb.tile([P, HALF], F32, tag="xn", bufs=2, name=f"xn2_{tcg}")
                    nc.gpsimd.dma_start(out=xn2,
                                        in_=xc[tcg * P:(tcg + 1) * P, HALF:DIM])
                    for jj in range(2):
                        pm = ps.tile([P, 512], F32, tag="pA", bufs=4,
                                     name=f"pg2_{tcg}_{jj}")
                        for k in range(KC):
                            nc.tensor.matmul(pm, lhsT=gts[k][:, t2 * P:(t2 + 1) * P],
                                             rhs=f2_t[k][:, jj * 512:(jj + 1) * 512],
                                             start=(k == 0),
                                             stop=(not with_bias and k == KC - 1))
                        if with_bias:
                            nc.tensor.matmul(pm, lhsT=ones_bf[0:1, :],
                                             rhs=b2r[0:1, jj * 512:(jj + 1) * 512],
                                             start=False, stop=True)
                        nc.vector.tensor_add(xn2[:, jj * 512:(jj + 1) * 512], pm,
                                             xn2[:, jj * 512:(jj + 1) * 512])
                    nc.scalar.dma_start(out=out[tcg * P:(tcg + 1) * P, HALF:DIM],
                                        in_=xn2)

            # ---------------- attention tail: vw, out-proj, y1 --------------
            rec = sb.tile([1, GRP], F32, tag="rec")
            nc.vector.reciprocal(rec, ndred[0:1, HALF:])
            vw_bf = sb.tile([1, HALF], BF, tag="vwbf")
            nc.vector.tensor_mul(
                vw_bf.rearrange("p (g c) -> p g c", c=4),
                ndred[0:1, 0:HALF].rearrange("p (g c) -> p g c", c=4),
                rec[0:1, :, None].to_broadcast([1, GRP, 4]))
            vwc = sb.tile([P, KC], BF, tag="vwc")
            for k in range(KC):
                pt = ps.tile([P, 1], F32, tag="pA", bufs=4, name=f"pvw{k}")
                nc.tensor.matmul(pt, lhsT=vw_bf[0:1, k * P:(k + 1) * P],
                                 rhs=ones_bf[0:1, 0:1], start=True, stop=True)
                nc.scalar.copy(vwc[:, k:k + 1], pt)
            orow = sb.tile([1, HALF], F32, tag="orow")
            for h in range(2):
                pm = ps.tile([1, 512], F32, tag="pB", bufs=4, name=f"po{h}")
                for k in range(KC):
                    nc.tensor.matmul(pm, lhsT=vwc[:, k:k + 1],
                                     rhs=wo_t[k][:, h * 512:(h + 1) * 512],
                                     start=(k == 0),
                                     stop=(not with_bias and k == KC - 1))
                if with_bias:
                    nc.tensor.matmul(pm, lhsT=ones_bf[0:1, 0:1],
                                     rhs=bor[0:1, h * 512:(h + 1) * 512],
                                     start=False, stop=True)
                nc.scalar.copy(orow[0:1, h * 512:(h + 1) * 512], pm)
            # broadcast out_row to 128 partitions, then DMA-accumulate it onto
            # the pre-copied x1 base (CCE add in the DMA datapath).
            obc = sb.tile([P, HALF], F32, tag="obc", name="obc")
            for h in range(2):
                pb = ps.tile([P, 512], F32, tag="pB", bufs=4, name=f"pbc{h}")
                nc.tensor.matmul(pb, lhsT=ones_f[0:1, :],
                                 rhs=orow[0:1, h * 512:(h + 1) * 512],
                                 start=True, stop=True)
                nc.scalar.copy(obc[:, h * 512:(h + 1) * 512], pb)
            acc = nc.gpsimd.dma_start(
                out=out[:, 0:HALF].rearrange("(a p) d -> p a d", p=P),
                in_=obc[:, None, :].to_broadcast([P, TC, HALF]),
                accum_op=ALU.add)
            tile.add_dep_helper(acc.ins, d2d.ins, sync=True,
                                reason="y1 accumulate after DRAM base copy")

    nc.compile()
    return nc


def _get_program(with_bias: bool):
    key = ("nc", with_bias)
    if key not in _CACHE:
        _CACHE[key] = _build_program(with_bias)
    return _CACHE[key]


def kernel(**inputs) -> np.ndarray:
    x = np.asarray(inputs["x"], np.float32)
    n1_g = np.asarray(inputs["n1_g"], np.float32)
    n1_b = np.asarray(inputs["n1_b"], np.float32)
    wq = np.asarray(inputs["wq"], np.float32)
    bq = np.asarray(inputs["bq"], np.float32)
    wk = np.asarray(inputs["wk"], np.float32)
    bk = np.asarray(inputs["bk"], np.float32)
    wv = np.asarray(inputs["wv"], np.float32)
    bv = np.asarray(inputs["bv"], np.float32)
    wo = np.asarray(inputs["wo"], np.float32)
    bo = np.asarray(inputs["bo"], np.float32)
    n2_g = np.asarray(inputs["n2_g"], np.float32)
    n2_b = np.asarray(inputs["n2_b"], np.float32)
    f1 = np.asarray(inputs["f1"], np.float32)
    b1 = np.asarray(inputs["b1"], np.float32)
    f2 = np.asarray(inputs["f2"], np.float32)
    b2 = np.asarray(inputs["b2"], np.float32)

    isr = 1.0 / math.sqrt(RANK)
    # fold LN affine: gamma into weight rows, beta into effective bias rows
    F1s = f1.sum(0)
    F2s = f2.sum(0)
    W1 = (n2_g[:, None] * F1s) * isr
    b1e = (n2_b @ F1s) * isr + b1
    Wqkv = np.concatenate([n1_g[:, None] * wq.T, n1_g[:, None] * wk.T,
                           n1_g[:, None] * wv.T], axis=1)
    bqkve = np.concatenate([n1_b @ wq.T + bq, n1_b @ wk.T + bk,
                            n1_b @ wv.T + bv])

    with_bias = bool(np.any(bqkve) or np.any(b2) or np.any(bo))

    wqkv_bf = Wqkv.astype(BF16)
    f1_bf = W1.astype(BF16)
    f2_bf = (F2s * isr).astype(BF16)
    woT_bf = np.ascontiguousarray(wo.T).astype(BF16)

    xf = np.ascontiguousarray(x.reshape(B * T, DIM))
    shared = {
        "wqkv": wqkv_bf,
        "f1w": f1_bf,
        "f2w": f2_bf,
        "woT": woT_bf,
        "b1e": np.ascontiguousarray(b1e.reshape(HALF, 1), dtype=np.float32),
    }
    if with_bias:
        shared["bqkve"] = np.ascontiguousarray(bqkve.reshape(1, -1)).astype(BF16)
        shared["b2e"] = np.ascontiguousarray(b2.reshape(1, -1)).astype(BF16)
        shared["boe"] = np.ascontiguousarray(bo.reshape(1, -1)).astype(BF16)
    in_maps = []
    for i in range(NCORES):
        rows = xf[i * ROWS:(i + 1) * ROWS]
        m = dict(shared)
        m["xc"] = rows
        m["xT"] = rows.T.astype(BF16, order="C")
        in_maps.append(m)

    nc = _get_program(with_bias)
    res = run_bass_kernel_spmd(nc, in_maps, core_ids=list(range(NCORES)))
    global _LAST_RESULTS
    _LAST_RESULTS = res
    y = np.concatenate([res.results[i]["out"] for i in range(NCORES)], axis=0)
    return np.ascontiguousarray(y.reshape(B, T, DIM))


# revision 41
# speedup vs baseline: 1.1297x; 1.1297x over previous
"""Trainium2 Bass kernel for nn_MicroStreamBlock (dual-stream block:
quaternion attention branch + Hamilton-mix MLP branch).

Contract: kernel(**inputs) takes the FULL unsharded inputs (as produced by
reference.setup_inputs) and returns the FULL (4, 2048, 2048) float32 output.
Internally the flattened (8192, 2048) token stream is row-sharded across 8
NeuronCores (1024 rows each; a core pair shares one batch).  The per-batch
softmax-over-time partial sums are combined with a tiny pairwise on-device
AllReduce that overlaps with the MLP-branch GEMMs.
"""

import math
import sys

sys.path.insert(0, "/opt/trn_rl_repo")

import ml_dtypes
import numpy as np

import concourse.bass as bass  # noqa: F401
import concourse.mybir as mybir
import concourse.tile as tile
from concourse import bacc
from concourse.bass_utils import run_bass_kernel_spmd

BF16 = ml_dtypes.bfloat16
F32 = mybir.dt.float32
BF = mybir.dt.bfloat16
AF = mybir.ActivationFunctionType
ALU = mybir.AluOpType
AX = mybir.AxisListType

NCORES = 8
B, T, DIM = 4, 2048, 2048
HALF = DIM // 2          # 1024
HEADS, RANK = 4, 8
NQ = (HALF // HEADS) // 4  # 64
GRP = HEADS * NQ           # 256 quaternion groups per token
ROWS = (B * T) // NCORES   # 1024 rows per core
P = 128
KC = HALF // P             # 8 contraction chunks of 128
TC = ROWS // P             # 8 token chunks of 128
LN_EPS = 1e-5
QEPS = 1e-24               # guard for ln of the quat-norm product

_CACHE: dict = {}
_LAST_RESULTS = None


def _build_program(with_bias: bool):
    nc = bacc.Bacc("TRN2", target_bir_lowering=False, debug=False,
                   num_devices=NCORES)

    xc = nc.dram_tensor("xc", [ROWS, DIM], F32, kind="ExternalInput").ap()
    xT = nc.dram_tensor("xT", [DIM, ROWS], BF, kind="ExternalInput").ap()
    wqkv_d = nc.dram_tensor("wqkv", [HALF, 3 * HALF], BF, kind="ExternalInput").ap()
    f1_d = nc.dram_tensor("f1w", [HALF, HALF], BF, kind="ExternalInput").ap()
    f2_d = nc.dram_tensor("f2w", [HALF, HALF], BF, kind="ExternalInput").ap()
    woT_d = nc.dram_tensor("woT", [HALF, HALF], BF, kind="ExternalInput").ap()
    b1_d = nc.dram_tensor("b1e", [HALF, 1], F32, kind="ExternalInput").ap()
    if with_bias:
        bqkv_d = nc.dram_tensor("bqkve", [1, 3 * HALF], BF, kind="ExternalInput").ap()
        b2_d = nc.dram_tensor("b2e", [1, HALF], BF, kind="ExternalInput").ap()
        bo_d = nc.dram_tensor("boe", [1, HALF], BF, kind="ExternalInput").ap()
    out = nc.dram_tensor("out", [ROWS, DIM], F32, kind="ExternalOutput").ap()

    with tile.TileContext(nc) as tc:
        with tc.tile_pool(name="sb", bufs=1) as sb, \
             tc.tile_pool(name="ps", bufs=1, space="PSUM") as ps, \
             tc.tile_pool(name="dp", bufs=1, space="DRAM") as dp:

            # ---------------- constants ----------------
            ones_bf = sb.tile([P, P], BF, tag="ones_bf")
            ones_f = sb.tile([P, P], F32, tag="ones_f")
            nc.vector.memset(ones_bf, 1.0)
            nc.vector.memset(ones_f, 1.0)
            epsln = sb.tile([P, 1], F32, tag="epsln")
            nc.vector.memset(epsln, LN_EPS)
            epsq = sb.tile([P, 1], F32, tag="epsq")
            nc.vector.memset(epsq, QEPS)

            b1cols = sb.tile([P, KC], F32, tag="b1cols")
            for k in range(KC):
                nc.sync.dma_start(out=b1cols[:, k:k + 1],
                                  in_=b1_d[k * P:(k + 1) * P, 0:1])
            if with_bias:
                bqkvr = sb.tile([1, 3 * HALF], BF, tag="bqkvr")
                nc.sync.dma_start(out=bqkvr, in_=bqkv_d)
                b2r = sb.tile([1, HALF], BF, tag="b2r")
                nc.sync.dma_start(out=b2r, in_=b2_d)
                bor = sb.tile([1, HALF], BF, tag="bor")
                nc.sync.dma_start(out=bor, in_=bo_d)

            # ---------------- persistent loads (x2 first: qkv needs it) ----
            x2t = []
            for k in range(KC):
                t2 = sb.tile([P, ROWS], BF, tag="xt", bufs=16, name=f"x2t{k}")
                nc.sync.dma_start(out=t2, in_=xT[HALF + k * P:HALF + (k + 1) * P, :])
                x2t.append(t2)
            x1t = []
            for k in range(KC):
                t1 = sb.tile([P, ROWS], BF, tag="xt", bufs=16, name=f"x1t{k}")
                nc.sync.dma_start(out=t1, in_=xT[k * P:(k + 1) * P, :])
                x1t.append(t1)
            wq_t = []
            for k in range(KC):
                t = sb.tile([P, 3 * HALF], BF, tag="w3", bufs=8, name=f"wq{k}")
                nc.sync.dma_start(out=t, in_=wqkv_d[k * P:(k + 1) * P, :])
                wq_t.append(t)
            f1_t = []
            for k in range(KC):
                t = sb.tile([P, HALF], BF, tag="wf", bufs=8, name=f"f1{k}")
                nc.sync.dma_start(out=t, in_=f1_d[k * P:(k + 1) * P, :])
                f1_t.append(t)
            f2_t = []
            for k in range(KC):
                t = sb.tile([P, HALF], BF, tag="wg", bufs=8, name=f"f2{k}")
                nc.sync.dma_start(out=t, in_=f2_d[k * P:(k + 1) * P, :])
                f2_t.append(t)

            # ---------------- LN stats from bf16 xT (PE column sums) -------
            # istd = exp(-0.5 ln(var+eps)); transient rows rotate 3 slots.
            def ln_stats(xt_tiles, label):
                psx = [ps.tile([1, 512], F32, tag="pB", bufs=4,
                               name=f"psx{label}{h}") for h in range(2)]
                psx2 = [ps.tile([1, 512], F32, tag="pB", bufs=4,
                                name=f"psx2{label}{h}") for h in range(2)]
                for k in range(KC):
                    sq = sb.tile([P, ROWS], BF, tag="sq", bufs=2, name=f"sq{label}{k}")
                    nc.vector.tensor_mul(sq, xt_tiles[k], xt_tiles[k])
                    for h in range(2):
                        nc.tensor.matmul(psx[h], lhsT=ones_bf[:, 0:1],
                                         rhs=xt_tiles[k][:, h * 512:(h + 1) * 512],
                                         start=(k == 0), stop=(k == KC - 1))
                        nc.tensor.matmul(psx2[h], lhsT=ones_bf[:, 0:1],
                                         rhs=sq[:, h * 512:(h + 1) * 512],
                                         start=(k == 0), stop=(k == KC - 1))
                m_row = sb.tile([1, ROWS], F32, tag="rowtmp", bufs=3,
                                name=f"m{label}")
                acc = sb.tile([1, ROWS], F32, tag="rowtmp", bufs=3,
                              name=f"acc{label}")
                for h in range(2):
                    nc.scalar.mul(m_row[0:1, h * 512:(h + 1) * 512], psx[h],
                                  1.0 / HALF)
                    nc.scalar.mul(acc[0:1, h * 512:(h + 1) * 512], psx2[h],
                                  1.0 / HALF)
                # acc: E[x^2] -> var -> ln(var+eps) -> istd   (in place)
                tmp = sb.tile([1, ROWS], F32, tag="rowtmp", bufs=3,
                              name=f"tmp{label}")
                nc.vector.tensor_mul(tmp, m_row, m_row)
                nc.vector.tensor_sub(acc, acc, tmp)
                nc.scalar.activation(acc, acc, AF.Ln, bias=epsln[0:1, 0:1])
                nc.scalar.activation(acc, acc, AF.Exp, scale=-0.5)
                return m_row, acc

            def bcast_row(row, name, half=None):
                """materialize (1,1024) f32 row -> (128,1024) bf16 tile"""
                bt = sb.tile([P, ROWS], BF, tag="bcast", bufs=2, name=name)
                for h in range(2):
                    pb = ps.tile([P, 512], F32, tag="pA", bufs=4,
                                 name=f"pb_{name}{h}")
                    nc.tensor.matmul(pb, lhsT=ones_f[0:1, :],
                                     rhs=row[0:1, h * 512:(h + 1) * 512],
                                     start=True, stop=True)
                    nc.scalar.copy(bt[:, h * 512:(h + 1) * 512], pb)
                return bt

            # stats for both branches first (keeps the PE column-sum matmuls
            # dense); row math overlaps on ACT/DVE
            m2_row, istd2_row = ln_stats(x2t, "b")
            m1_row, istd1_row = ln_stats(x1t, "a")

            # branch f (x2): center in place; istd2 applied at qkv eviction
            m2bc = bcast_row(m2_row, "m2bc")
            for k in range(KC):
                nc.vector.tensor_sub(x2t[k], x2t[k], m2bc)
            istd2c = sb.tile([P, TC], F32, tag="istd2c")
            for c in range(TC):
                pt = ps.tile([P, 1], F32, tag="pA", bufs=4, name=f"ptr{c}")
                nc.tensor.matmul(pt, lhsT=istd2_row[0:1, c * P:(c + 1) * P],
                                 rhs=ones_f[0:1, 0:1], start=True, stop=True)
                nc.scalar.copy(istd2c[:, c:c + 1], pt)
            std2_bf = None
            if with_bias:
                std2_row = sb.tile([1, ROWS], F32, tag="std2row")
                nc.vector.reciprocal(std2_row, istd2_row)
                std2_bf = sb.tile([1, ROWS], BF, tag="std2bf")
                nc.vector.tensor_copy(std2_bf, std2_row)

            # branch g (x1): full normalize in place (istd1 is per-free in
            # the transposed f1 GEMM, so it must be pre-applied)
            m1bc = bcast_row(m1_row, "m1bc")
            istd1bc = bcast_row(istd1_row, "istd1bc")
            for k in range(KC):
                nc.vector.tensor_sub(x1t[k], x1t[k], m1bc)
                nc.vector.tensor_mul(x1t[k], x1t[k], istd1bc)

            # ---------------- stage 1: qkv GEMM + attention partials -------
            nd = [ps.tile([1, 512], F32, tag="pB", bufs=4, name="nd0"),
                  ps.tile([1, 512], F32, tag="pB", bufs=4, name="nd1"),
                  ps.tile([1, 256], F32, tag="pB", bufs=4, name="nd2")]
            nd_slices = [(0, 512), (512, 512), (1024, 256)]

            wds = []
            for c in range(TC):
                q = sb.tile([P, HALF], BF, tag="qk", bufs=3, name=f"q{c}")
                kk_t = sb.tile([P, HALF], BF, tag="qk", bufs=3, name=f"k{c}")
                v = sb.tile([P, HALF], BF, tag="vv", bufs=2, name=f"v{c}")
                dests = [(q, 0), (q, 512), (kk_t, 0), (kk_t, 512), (v, 0), (v, 512)]
                for j in range(6):
                    pm = ps.tile([P, 512], F32, tag="pA", bufs=4, name=f"pqkv{c}_{j}")
                    for k in range(KC):
                        nc.tensor.matmul(pm, lhsT=x2t[k][:, c * P:(c + 1) * P],
                                         rhs=wq_t[k][:, j * 512:(j + 1) * 512],
                                         start=(k == 0),
                                         stop=(not with_bias and k == KC - 1))
                    if with_bias:
                        # psum += outer(std2(t), b(j)); istd2 at eviction
                        # turns this into the plain + b(j).
                        nc.tensor.matmul(pm, lhsT=std2_bf[0:1, c * P:(c + 1) * P],
                                         rhs=bqkvr[0:1, j * 512:(j + 1) * 512],
                                         start=False, stop=True)
                    dt, off = dests[j]
                    nc.scalar.mul(dt[:, off:off + 512], pm, istd2c[:, c:c + 1])

                # quaternion products -> group sums over the 4-vector
                prod = sb.tile([P, HALF], BF, tag="sq", bufs=2, name=f"pr{c}")
                sqq = sb.tile([P, GRP], F32, tag="ss", bufs=3, name=f"sqq{c}")
                skk = sb.tile([P, GRP], F32, tag="ss", bufs=3, name=f"skk{c}")
                sqk = sb.tile([P, GRP], F32, tag="ss", bufs=3, name=f"sqk{c}")
                nc.vector.tensor_mul(prod, q, q)
                nc.vector.tensor_reduce(sqq, prod.rearrange("p (g c) -> p g c", c=4),
                                        axis=AX.X, op=ALU.add)
                nc.vector.tensor_mul(prod, kk_t, kk_t)
                nc.vector.tensor_reduce(skk, prod.rearrange("p (g c) -> p g c", c=4),
                                        axis=AX.X, op=ALU.add)
                nc.vector.tensor_mul(prod, q, kk_t)
                nc.vector.tensor_reduce(sqk, prod.rearrange("p (g c) -> p g c", c=4),
                                        axis=AX.X, op=ALU.add)

                # l = sqk/sqrt(sqq*skk);  e = exp(l/8)   (ln/exp only)
                nc.vector.tensor_mul(sqq, sqq, skk)
                nc.scalar.activation(sqq, sqq, AF.Ln, bias=epsq)
                nc.scalar.activation(sqq, sqq, AF.Exp, scale=-0.5)
                nc.vector.tensor_mul(sqk, sqk, sqq)
                wd = sb.tile([P, HALF + GRP], BF, tag="wd", bufs=3, name=f"wd{c}")
                nc.scalar.activation(wd[:, HALF:], sqk, AF.Exp,
                                     scale=1.0 / math.sqrt(NQ))
                wd_mul_inst = nc.vector.tensor_mul(
                    wd[:, 0:HALF].rearrange("p (g c) -> p g c", c=4),
                    v.rearrange("p (g c) -> p g c", c=4),
                    wd[:, HALF:][:, :, None].to_broadcast([P, GRP, 4]))
                wds.append(wd)
                if c == 3:
                    dep_anchor = wd_mul_inst
                # numerator/denominator accumulation, deferred one chunk so the
                # attention vector chain stays off the PE critical path
                if c >= 1:
                    for s, (lo, n) in enumerate(nd_slices):
                        nc.tensor.matmul(nd[s], lhsT=ones_bf[:, 0:1],
                                         rhs=wds[c - 1][:, lo:lo + n],
                                         start=(c == 1), stop=False,
                                         skip_group_check=True)
            for s, (lo, n) in enumerate(nd_slices):
                nc.tensor.matmul(nd[s], lhsT=ones_bf[:, 0:1],
                                 rhs=wds[TC - 1][:, lo:lo + n],
                                 start=False, stop=True,
                                 skip_group_check=True)

            # ---------------- pairwise AllReduce of [num | den] -------------
            ndrow = sb.tile([1, HALF + GRP], F32, tag="ndrow")
            for s, (lo, n) in enumerate(nd_slices):
                nc.scalar.copy(ndrow[0:1, lo:lo + n], nd[s])
            ndin = dp.tile([1, HALF + GRP], F32, tag="ndin")
            ndout = dp.tile([1, HALF + GRP], F32, tag="ndout")
            nc.sync.dma_start(out=ndin, in_=ndrow)
            nc.gpsimd.collective_compute(
                "AllReduce", ALU.add,
                replica_groups=[[0, 1], [2, 3], [4, 5], [6, 7]],
                ins=[ndin.opt()], outs=[ndout.opt()])
            ndred = sb.tile([1, HALF + GRP], F32, tag="ndred")
            nc.sync.dma_start(out=ndred, in_=ndout)

            # y1 base copy (DRAM->DRAM) and the out-proj weights.  Both are
            # dep-free, so without a throttle the DMA queues would hoist them
            # to t=0 and starve the critical x2T/wqkv loads; anchor them to
            # mid-stage-1 instead (their results are needed only ~200us in).
            d2d = nc.gpsimd.dma_start(out=out[:, 0:HALF], in_=xc[:, 0:HALF])
            tile.add_dep_helper(d2d.ins, dep_anchor.ins, sync=True,
                                reason="defer y1 base copy past early loads")
            wo_t = []
            for k in range(KC):
                t = sb.tile([P, HALF], BF, tag="wo", bufs=8, name=f"wo{k}")
                ld = nc.gpsimd.dma_start(out=t, in_=woT_d[k * P:(k + 1) * P, :])
                if k == 0:
                    tile.add_dep_helper(ld.ins, dep_anchor.ins, sync=True,
                                        reason="defer wo loads past early loads")
                wo_t.append(t)

            # ---------------- stage 2: Hamilton-mix branch ------------------
            for tt in range(2):
                gts = []
                for jc in range(KC):
                    pm = ps.tile([P, 512], F32, tag="pA", bufs=4,
                                 name=f"pg1_{tt}_{jc}")
                    for k in range(KC):
                        nc.tensor.matmul(pm, lhsT=f1_t[k][:, jc * P:(jc + 1) * P],
                                         rhs=x1t[k][:, tt * 512:(tt + 1) * 512],
                                         start=(k == 0), stop=(k == KC - 1))
                    gt = sb.tile([P, 512], BF, tag="gt", bufs=8, name=f"gt{tt}_{jc}")
                    nc.scalar.activation(gt, pm, AF.Gelu, bias=b1cols[:, jc:jc + 1])
                    gts.append(gt)
                for t2 in range(4):
                    tcg = tt * 4 + t2
                    xn2 = sb.tile([P, HALF], F32, tag="xn", bufs=2, name=f"xn2_{tcg}")
                    nc.sync.dma_start(out=xn2,
                                      in_=xc[tcg * P:(tcg + 1) * P, HALF:DIM])
                    for jj in range(2):
                        pm = ps.tile([P, 512], F32, tag="pA", bufs=4,
                                     name=f"pg2_{tcg}_{jj}")
                        for k in range(KC):
                            nc.tensor.matmul(pm, lhsT=gts[k][:, t2 * P:(t2 + 1) * P],
                                             rhs=f2_t[k][:, jj * 512:(jj + 1) * 512],
                                             start=(k == 0),
                                             stop=(not with_bias and k == KC - 1))
                        if with_bias:
                            nc.tensor.matmul(pm, lhsT=ones_bf[0:1, :],
                                             rhs=b2r[0:1, jj * 512:(jj + 1) * 512],
                                             start=False, stop=True)
                        nc.vector.tensor_add(xn2[:, jj * 512:(jj + 1) * 512], pm,
                                             xn2[:, jj * 512:(jj + 1) * 512])
                    nc.scalar.dma_start(out=out[tcg * P:(tcg + 1) * P, HALF:DIM],
                                        in_=xn2)

            # ---------------- attention tail: vw, out-proj, y1 --------------
            rec = sb.tile([1, GRP], F32, tag="rec")
            nc.vector.reciprocal(rec, ndred[0:1, HALF:])
            vw_bf = sb.tile([1, HALF], BF, tag="vwbf")
            nc.vector.tensor_mul(
                vw_bf.rearrange("p (g c) -> p g c", c=4),
                ndred[0:1, 0:HALF].rearrange("p (g c) -> p g c", c=4),
                rec[0:1, :, None].to_broadcast([1, GRP, 4]))
            vwc = sb.tile([P, KC], BF, tag="vwc")
            for k in range(KC):
                pt = ps.tile([P, 1], F32, tag="pA", bufs=4, name=f"pvw{k}")
                nc.tensor.matmul(pt, lhsT=vw_bf[0:1, k * P:(k + 1) * P],
                                 rhs=ones_bf[0:1, 0:1], start=True, stop=True)
                nc.scalar.copy(vwc[:, k:k + 1], pt)
            orow = sb.tile([1, HALF], F32, tag="orow")
            for h in range(2):
                pm = ps.tile([1, 512], F32, tag="pB", bufs=4, name=f"po{h}")
                for k in range(KC):
                    nc.tensor.matmul(pm, lhsT=vwc[:, k:k + 1],
                                     rhs=wo_t[k][:, h * 512:(h + 1) * 512],
                                     start=(k == 0),
                                     stop=(not with_bias and k == KC - 1))
                if with_bias:
                    nc.tensor.matmul(pm, lhsT=ones_bf[0:1, 0:1],
                                     rhs=bor[0:1, h * 512:(h + 1) * 512],
                                     start=False, stop=True)
                nc.scalar.copy(orow[0:1, h * 512:(h + 1) * 512], pm)
            # broadcast out_row to 128 partitions, then DMA-accumulate it onto
            # the pre-copied x1 base (CCE add in the DMA datapath).
            obc = sb.tile([P, HALF], F32, tag="obc", name="obc")
            for h in range(2):
                pb = ps.tile([P, 512], F32, tag="pB", bufs=4, name=f"pbc{h}")
                nc.tensor.matmul(pb, lhsT=ones_f[0:1, :],
                                 rhs=orow[0:1, h * 512:(h + 1) * 512],
                                 start=True, stop=True)
                nc.scalar.copy(obc[:, h * 512:(h + 1) * 512], pb)
            acc = nc.gpsimd.dma_start(
                out=out[:, 0:HALF].rearrange("(a p) d -> p a d", p=P),
                in_=obc[:, None, :].to_broadcast([P, TC, HALF]),
                accum_op=ALU.add)
            tile.add_dep_helper(acc.ins, d2d.ins, sync=True,
                                reason="y1 accumulate after DRAM base copy")

    nc.compile()
    return nc


def _get_program(with_bias: bool):
    key = ("nc", with_bias)
    if key not in _CACHE:
        _CACHE[key] = _build_program(with_bias)
    return _CACHE[key]


def kernel(**inputs) -> np.ndarray:
    x = np.asarray(inputs["x"], np.float32)
    n1_g = np.asarray(inputs["n1_g"], np.float32)
    n1_b = np.asarray(inputs["n1_b"], np.float32)
    wq = np.asarray(inputs["wq"], np.float32)
    bq = np.asarray(inputs["bq"], np.float32)
    wk = np.asarray(inputs["wk"], np.float32)
    bk = np.asarray(inputs["bk"], np.float32)
    wv = np.asarray(inputs["wv"], np.float32)
    bv = np.asarray(inputs["bv"], np.float32)
    wo = np.asarray(inputs["wo"], np.float32)
    bo = np.asarray(inputs["bo"], np.float32)
    n2_g = np.asarray(inputs["n2_g"], np.float32)
    n2_b = np.asarray(inputs["n2_b"], np.float32)
    f1 = np.asarray(inputs["f1"], np.float32)
    b1 = np.asarray(inputs["b1"], np.float32)
    f2 = np.asarray(inputs["f2"], np.float32)
    b2 = np.asarray(inputs["b2"], np.float32)

    isr = 1.0 / math.sqrt(RANK)
    # fold LN affine: gamma into weight rows, beta into effective bias rows
    F1s = f1.sum(0)
    F2s = f2.sum(0)
    W1 = (n2_g[:, None] * F1s) * isr
    b1e = (n2_b @ F1s) * isr + b1
    Wqkv = np.concatenate([n1_g[:, None] * wq.T, n1_g[:, None] * wk.T,
                           n1_g[:, None] * wv.T], axis=1)
    bqkve = np.concatenate([n1_b @ wq.T + bq, n1_b @ wk.T + bk,
                            n1_b @ wv.T + bv])

    with_bias = bool(np.any(bqkve) or np.any(b2) or np.any(bo))

    wqkv_bf = Wqkv.astype(BF16)
    f1_bf = W1.astype(BF16)
    f2_bf = (F2s * isr).astype(BF16)
    woT_bf = np.ascontiguousarray(wo.T).astype(BF16)

    xf = np.ascontiguousarray(x.reshape(B * T, DIM))
    shared = {
        "wqkv": wqkv_bf,
        "f1w": f1_bf,
        "f2w": f2_bf,
        "woT": woT_bf,
        "b1e": np.ascontiguousarray(b1e.reshape(HALF, 1), dtype=np.float32),
    }
    if with_bias:
        shared["bqkve"] = np.ascontiguousarray(bqkve.reshape(1, -1)).astype(BF16)
        shared["b2e"] = np.ascontiguousarray(b2.reshape(1, -1)).astype(BF16)
        shared["boe"] = np.ascontiguousarray(bo.reshape(1, -1)).astype(BF16)
    in_maps = []
    for i in range(NCORES):
        rows = xf[i * ROWS:(i + 1) * ROWS]
        m = dict(shared)
        m["xc"] = rows
        m["xT"] = rows.T.astype(BF16, order="C")
        in_maps.append(m)

    nc = _get_program(with_bias)
    res = run_bass_kernel_spmd(nc, in_maps, core_ids=list(range(NCORES)))
    global _LAST_RESULTS
    _LAST_RESULTS = res
    y = np.concatenate([res.results[i]["out"] for i in range(NCORES)], axis=0)
    return np.ascontiguousarray(y.reshape(B, T, DIM))


# revision 45
# speedup vs baseline: 1.2120x; 1.0728x over previous
"""Trainium2 Bass kernel for nn_MicroStreamBlock (dual-stream block:
quaternion attention branch + Hamilton-mix MLP branch).

Contract: kernel(**inputs) takes the FULL unsharded inputs (as produced by
reference.setup_inputs) and returns the FULL (4, 2048, 2048) float32 output.
Internally the flattened (8192, 2048) token stream is row-sharded across 8
NeuronCores (1024 rows each; a core pair shares one batch).  The per-batch
softmax-over-time partial sums are combined with a tiny pairwise on-device
AllReduce that overlaps with the MLP-branch GEMMs.
"""

import math
import sys

sys.path.insert(0, "/opt/trn_rl_repo")

import ml_dtypes
import numpy as np

import concourse.bass as bass  # noqa: F401
import concourse.mybir as mybir
import concourse.tile as tile
from concourse import bacc
from concourse.bass_utils import run_bass_kernel_spmd

BF16 = ml_dtypes.bfloat16
F32 = mybir.dt.float32
BF = mybir.dt.bfloat16
AF = mybir.ActivationFunctionType
ALU = mybir.AluOpType
AX = mybir.AxisListType

NCORES = 8
B, T, DIM = 4, 2048, 2048
HALF = DIM // 2          # 1024
HEADS, RANK = 4, 8
NQ = (HALF // HEADS) // 4  # 64
GRP = HEADS * NQ           # 256 quaternion groups per token
ROWS = (B * T) // NCORES   # 1024 rows per core
P = 128
KC = HALF // P             # 8 contraction chunks of 128
TC = ROWS // P             # 8 token chunks of 128
LN_EPS = 1e-5
QEPS = 1e-24               # guard for ln of the quat-norm product

_CACHE: dict = {}
_LAST_RESULTS = None


def _build_program(with_bias: bool):
    nc = bacc.Bacc("TRN2", target_bir_lowering=False, debug=False,
                   num_devices=NCORES)

    xc = nc.dram_tensor("xc", [ROWS, DIM], F32, kind="ExternalInput").ap()
    xT = nc.dram_tensor("xT", [DIM, ROWS], BF, kind="ExternalInput").ap()
    wqkv_d = nc.dram_tensor("wqkv", [HALF, 3 * HALF], BF, kind="ExternalInput").ap()
    f1_d = nc.dram_tensor("f1w", [HALF, HALF], BF, kind="ExternalInput").ap()
    f2_d = nc.dram_tensor("f2w", [HALF, HALF], BF, kind="ExternalInput").ap()
    woT_d = nc.dram_tensor("woT", [HALF, HALF], BF, kind="ExternalInput").ap()
    b1_d = nc.dram_tensor("b1e", [HALF, 1], F32, kind="ExternalInput").ap()
    if with_bias:
        bqkv_d = nc.dram_tensor("bqkve", [1, 3 * HALF], BF, kind="ExternalInput").ap()
        b2_d = nc.dram_tensor("b2e", [1, HALF], BF, kind="ExternalInput").ap()
        bo_d = nc.dram_tensor("boe", [1, HALF], BF, kind="ExternalInput").ap()
    out = nc.dram_tensor("out", [ROWS, DIM], F32, kind="ExternalOutput").ap()

    with tile.TileContext(nc) as tc:
        with tc.tile_pool(name="sb", bufs=1) as sb, \
             tc.tile_pool(name="ps", bufs=1, space="PSUM") as ps, \
             tc.tile_pool(name="dp", bufs=1, space="DRAM") as dp:

            # ---------------- constants ----------------
            ones_bf = sb.tile([P, P], BF, tag="ones_bf")
            ones_f = sb.tile([P, P], F32, tag="ones_f")
            nc.vector.memset(ones_bf, 1.0)
            nc.vector.memset(ones_f, 1.0)
            epsln = sb.tile([P, 1], F32, tag="epsln")
            nc.vector.memset(epsln, LN_EPS)
            epsq = sb.tile([P, 1], F32, tag="epsq")
            nc.vector.memset(epsq, QEPS)

            b1cols = sb.tile([P, KC], F32, tag="b1cols")
            for k in range(KC):
                nc.sync.dma_start(out=b1cols[:, k:k + 1],
                                  in_=b1_d[k * P:(k + 1) * P, 0:1])
            if with_bias:
                bqkvr = sb.tile([1, 3 * HALF], BF, tag="bqkvr")
                nc.sync.dma_start(out=bqkvr, in_=bqkv_d)
                b2r = sb.tile([1, HALF], BF, tag="b2r")
                nc.sync.dma_start(out=b2r, in_=b2_d)
                bor = sb.tile([1, HALF], BF, tag="bor")
                nc.sync.dma_start(out=bor, in_=bo_d)

            # ---------------- persistent loads (x2 first: qkv needs it) ----
            x2t = []
            for k in range(KC):
                t2 = sb.tile([P, ROWS], BF, tag="xt", bufs=16, name=f"x2t{k}")
                nc.sync.dma_start(out=t2, in_=xT[HALF + k * P:HALF + (k + 1) * P, :])
                x2t.append(t2)
            x1t = []
            for k in range(KC):
                t1 = sb.tile([P, ROWS], BF, tag="xt", bufs=16, name=f"x1t{k}")
                nc.sync.dma_start(out=t1, in_=xT[k * P:(k + 1) * P, :])
                x1t.append(t1)
            wq_t = []
            for k in range(KC):
                t = sb.tile([P, 3 * HALF], BF, tag="w3", bufs=8, name=f"wq{k}")
                nc.sync.dma_start(out=t, in_=wqkv_d[k * P:(k + 1) * P, :])
                wq_t.append(t)
            f1_t = []
            for k in range(KC):
                t = sb.tile([P, HALF], BF, tag="wf", bufs=8, name=f"f1{k}")
                nc.sync.dma_start(out=t, in_=f1_d[k * P:(k + 1) * P, :])
                f1_t.append(t)
            f2_t = []
            for k in range(KC):
                t = sb.tile([P, HALF], BF, tag="wg", bufs=8, name=f"f2{k}")
                nc.sync.dma_start(out=t, in_=f2_d[k * P:(k + 1) * P, :])
                f2_t.append(t)

            # ---------------- LN stats from bf16 xT (PE column sums) -------
            # istd = exp(-0.5 ln(var+eps)); transient rows rotate 3 slots.
            def ln_stats(xt_tiles, label):
                psx = [ps.tile([1, 512], F32, tag="pB", bufs=4,
                               name=f"psx{label}{h}") for h in range(2)]
                psx2 = [ps.tile([1, 512], F32, tag="pB", bufs=4,
                                name=f"psx2{label}{h}") for h in range(2)]
                for k in range(KC):
                    sq = sb.tile([P, ROWS], BF, tag="sq", bufs=2, name=f"sq{label}{k}")
                    nc.vector.tensor_mul(sq, xt_tiles[k], xt_tiles[k])
                    for h in range(2):
                        nc.tensor.matmul(psx[h], lhsT=ones_bf[:, 0:1],
                                         rhs=xt_tiles[k][:, h * 512:(h + 1) * 512],
                                         start=(k == 0), stop=(k == KC - 1))
                        nc.tensor.matmul(psx2[h], lhsT=ones_bf[:, 0:1],
                                         rhs=sq[:, h * 512:(h + 1) * 512],
                                         start=(k == 0), stop=(k == KC - 1))
                m_row = sb.tile([1, ROWS], F32, tag="rowtmp", bufs=3,
                                name=f"m{label}")
                acc = sb.tile([1, ROWS], F32, tag="rowtmp", bufs=3,
                              name=f"acc{label}")
                for h in range(2):
                    nc.scalar.mul(m_row[0:1, h * 512:(h + 1) * 512], psx[h],
                                  1.0 / HALF)
                    nc.scalar.mul(acc[0:1, h * 512:(h + 1) * 512], psx2[h],
                                  1.0 / HALF)
                # acc: E[x^2] -> var -> ln(var+eps) -> istd   (in place)
                tmp = sb.tile([1, ROWS], F32, tag="rowtmp", bufs=3,
                              name=f"tmp{label}")
                nc.vector.tensor_mul(tmp, m_row, m_row)
                nc.vector.tensor_sub(acc, acc, tmp)
                nc.scalar.activation(acc, acc, AF.Ln, bias=epsln[0:1, 0:1])
                nc.scalar.activation(acc, acc, AF.Exp, scale=-0.5)
                return m_row, acc

            def bcast_row(row, name, half=None):
                """materialize (1,1024) f32 row -> (128,1024) bf16 tile"""
                bt = sb.tile([P, ROWS], BF, tag="bcast", bufs=2, name=name)
                for h in range(2):
                    pb = ps.tile([P, 512], F32, tag="pA", bufs=4,
                                 name=f"pb_{name}{h}")
                    nc.tensor.matmul(pb, lhsT=ones_f[0:1, :],
                                     rhs=row[0:1, h * 512:(h + 1) * 512],
                                     start=True, stop=True)
                    nc.scalar.copy(bt[:, h * 512:(h + 1) * 512], pb)
                return bt

            # stats for both branches first (keeps the PE column-sum matmuls
            # dense); row math overlaps on ACT/DVE
            m2_row, istd2_row = ln_stats(x2t, "b")
            m1_row, istd1_row = ln_stats(x1t, "a")

            # branch f (x2): center in place; istd2 applied at qkv eviction
            m2bc = bcast_row(m2_row, "m2bc")
            for k in range(KC):
                nc.vector.tensor_sub(x2t[k], x2t[k], m2bc)
            istd2c = sb.tile([P, TC], F32, tag="istd2c")
            for c in range(TC):
                pt = ps.tile([P, 1], F32, tag="pA", bufs=4, name=f"ptr{c}")
                nc.tensor.matmul(pt, lhsT=istd2_row[0:1, c * P:(c + 1) * P],
                                 rhs=ones_f[0:1, 0:1], start=True, stop=True)
                nc.scalar.copy(istd2c[:, c:c + 1], pt)
            std2_bf = None
            if with_bias:
                std2_row = sb.tile([1, ROWS], F32, tag="std2row")
                nc.vector.reciprocal(std2_row, istd2_row)
                std2_bf = sb.tile([1, ROWS], BF, tag="std2bf")
                nc.vector.tensor_copy(std2_bf, std2_row)

            # branch g (x1): full normalize in place (istd1 is per-free in
            # the transposed f1 GEMM, so it must be pre-applied)
            m1bc = bcast_row(m1_row, "m1bc")
            istd1bc = bcast_row(istd1_row, "istd1bc")
            for k in range(KC):
                nc.vector.tensor_sub(x1t[k], x1t[k], m1bc)
                nc.vector.tensor_mul(x1t[k], x1t[k], istd1bc)

            # ---------------- stage 1: qkv GEMM + attention partials -------
            nd = [ps.tile([1, 512], F32, tag="pB", bufs=4, name="nd0"),
                  ps.tile([1, 512], F32, tag="pB", bufs=4, name="nd1"),
                  ps.tile([1, 256], F32, tag="pB", bufs=4, name="nd2")]
            nd_slices = [(0, 512), (512, 512), (1024, 256)]

            wds = []
            for c in range(TC):
                q = sb.tile([P, HALF], BF, tag="qk", bufs=3, name=f"q{c}")
                kk_t = sb.tile([P, HALF], BF, tag="qk", bufs=3, name=f"k{c}")
                v = sb.tile([P, HALF], BF, tag="vv", bufs=2, name=f"v{c}")
                dests = [(q, 0), (q, 512), (kk_t, 0), (kk_t, 512), (v, 0), (v, 512)]
                for j in range(6):
                    pm = ps.tile([P, 512], F32, tag="pA", bufs=4, name=f"pqkv{c}_{j}")
                    for k in range(KC):
                        nc.tensor.matmul(pm, lhsT=x2t[k][:, c * P:(c + 1) * P],
                                         rhs=wq_t[k][:, j * 512:(j + 1) * 512],
                                         start=(k == 0),
                                         stop=(not with_bias and k == KC - 1))
                    if with_bias:
                        # psum += outer(std2(t), b(j)); istd2 at eviction
                        # turns this into the plain + b(j).
                        nc.tensor.matmul(pm, lhsT=std2_bf[0:1, c * P:(c + 1) * P],
                                         rhs=bqkvr[0:1, j * 512:(j + 1) * 512],
                                         start=False, stop=True)
                    dt, off = dests[j]
                    nc.scalar.mul(dt[:, off:off + 512], pm, istd2c[:, c:c + 1])

                # quaternion products -> group sums over the 4-vector
                prod = sb.tile([P, HALF], BF, tag="sq", bufs=2, name=f"pr{c}")
                sqq = sb.tile([P, GRP], F32, tag="ss", bufs=3, name=f"sqq{c}")
                skk = sb.tile([P, GRP], F32, tag="ss", bufs=3, name=f"skk{c}")
                sqk = sb.tile([P, GRP], F32, tag="ss", bufs=3, name=f"sqk{c}")
                nc.vector.tensor_mul(prod, q, q)
                nc.vector.tensor_reduce(sqq, prod.rearrange("p (g c) -> p g c", c=4),
                                        axis=AX.X, op=ALU.add)
                nc.vector.tensor_mul(prod, kk_t, kk_t)
                nc.vector.tensor_reduce(skk, prod.rearrange("p (g c) -> p g c", c=4),
                                        axis=AX.X, op=ALU.add)
                nc.vector.tensor_mul(prod, q, kk_t)
                nc.vector.tensor_reduce(sqk, prod.rearrange("p (g c) -> p g c", c=4),
                                        axis=AX.X, op=ALU.add)

                # l = sqk/sqrt(sqq*skk);  e = exp(l/8)   (ln/exp only)
                nc.vector.tensor_mul(sqq, sqq, skk)
                nc.scalar.activation(sqq, sqq, AF.Ln, bias=epsq)
                nc.scalar.activation(sqq, sqq, AF.Exp, scale=-0.5)
                nc.vector.tensor_mul(sqk, sqk, sqq)
                wd = sb.tile([P, HALF + GRP], BF, tag="wd", bufs=3, name=f"wd{c}")
                nc.scalar.activation(wd[:, HALF:], sqk, AF.Exp,
                                     scale=1.0 / math.sqrt(NQ))
                wd_mul_inst = nc.vector.tensor_mul(
                    wd[:, 0:HALF].rearrange("p (g c) -> p g c", c=4),
                    v.rearrange("p (g c) -> p g c", c=4),
                    wd[:, HALF:][:, :, None].to_broadcast([P, GRP, 4]))
                wds.append(wd)
                if c == 3:
                    dep_anchor = wd_mul_inst
                # numerator/denominator accumulation, deferred one chunk so the
                # attention vector chain stays off the PE critical path
                if c >= 1:
                    for s, (lo, n) in enumerate(nd_slices):
                        nc.tensor.matmul(nd[s], lhsT=ones_bf[:, 0:1],
                                         rhs=wds[c - 1][:, lo:lo + n],
                                         start=(c == 1), stop=False,
                                         skip_group_check=True)
            for s, (lo, n) in enumerate(nd_slices):
                nc.tensor.matmul(nd[s], lhsT=ones_bf[:, 0:1],
                                 rhs=wds[TC - 1][:, lo:lo + n],
                                 start=False, stop=True,
                                 skip_group_check=True)

            # ---------------- pairwise AllReduce of [num | den] -------------
            ndrow = sb.tile([1, HALF + GRP], F32, tag="ndrow")
            for s, (lo, n) in enumerate(nd_slices):
                nc.scalar.copy(ndrow[0:1, lo:lo + n], nd[s])
            ndin = dp.tile([1, HALF + GRP], F32, tag="ndin")
            ndout = dp.tile([1, HALF + GRP], F32, tag="ndout")
            nc.sync.dma_start(out=ndin, in_=ndrow)
            nc.gpsimd.collective_compute(
                "AllReduce", ALU.add,
                replica_groups=[[0, 1], [2, 3], [4, 5], [6, 7]],
                ins=[ndin.opt()], outs=[ndout.opt()])
            ndred = sb.tile([1, HALF + GRP], F32, tag="ndred")
            nc.sync.dma_start(out=ndred, in_=ndout)

            # y1 base copy (DRAM->DRAM) and the out-proj weights.  Both are
            # dep-free, so without a throttle the DMA queues would hoist them
            # to t=0 and starve the critical x2T/wqkv loads; anchor them to
            # mid-stage-1 instead (their results are needed only ~200us in).
            d2d = nc.gpsimd.dma_start(out=out[:, 0:HALF], in_=xc[:, 0:HALF])
            tile.add_dep_helper(d2d.ins, dep_anchor.ins, sync=True,
                                reason="defer y1 base copy past early loads")
            wo_t = []
            for k in range(KC):
                t = sb.tile([P, HALF], BF, tag="wo", bufs=8, name=f"wo{k}")
                ld = nc.gpsimd.dma_start(out=t, in_=woT_d[k * P:(k + 1) * P, :])
                if k == 0:
                    tile.add_dep_helper(ld.ins, dep_anchor.ins, sync=True,
                                        reason="defer wo loads past early loads")
                wo_t.append(t)

            # ---------------- stage 2: Hamilton-mix branch ------------------
            for tt in range(2):
                gts = []
                for jc in range(KC):
                    pm = ps.tile([P, 512], F32, tag="pA", bufs=4,
                                 name=f"pg1_{tt}_{jc}")
                    for k in range(KC):
                        nc.tensor.matmul(pm, lhsT=f1_t[k][:, jc * P:(jc + 1) * P],
                                         rhs=x1t[k][:, tt * 512:(tt + 1) * 512],
                                         start=(k == 0), stop=(k == KC - 1))
                    gt = sb.tile([P, 512], BF, tag="gt", bufs=8, name=f"gt{tt}_{jc}")
                    nc.scalar.activation(gt, pm, AF.Gelu, bias=b1cols[:, jc:jc + 1])
                    gts.append(gt)
                for t2 in range(4):
                    tcg = tt * 4 + t2
                    xn2 = sb.tile([P, HALF], F32, tag="xn", bufs=2, name=f"xn2_{tcg}")
                    nc.sync.dma_start(out=xn2,
                                      in_=xc[tcg * P:(tcg + 1) * P, HALF:DIM])
                    for jj in range(2):
                        pm = ps.tile([P, 512], F32, tag="pA", bufs=4,
                                     name=f"pg2_{tcg}_{jj}")
                        for k in range(KC):
                            last_mm = nc.tensor.matmul(
                                pm, lhsT=gts[k][:, t2 * P:(t2 + 1) * P],
                                rhs=f2_t[k][:, jj * 512:(jj + 1) * 512],
                                start=(k == 0),
                                stop=(not with_bias and k == KC - 1))
                        if with_bias:
                            nc.tensor.matmul(pm, lhsT=ones_bf[0:1, :],
                                             rhs=b2r[0:1, jj * 512:(jj + 1) * 512],
                                             start=False, stop=True)
                        last_add = nc.vector.tensor_add(
                            xn2[:, jj * 512:(jj + 1) * 512], pm,
                            xn2[:, jj * 512:(jj + 1) * 512])
                    nc.scalar.dma_start(out=out[tcg * P:(tcg + 1) * P, HALF:DIM],
                                        in_=xn2)

            # ---------------- attention tail: vw, out-proj, y1 --------------
            # The collective's real latency isn't modeled by the scheduler;
            # keep the whole tail behind stage 2 in the engine FIFOs so a
            # long collective can't block the residual adds / PSUM recycling.
            rec = sb.tile([1, GRP], F32, tag="rec")
            rec_i = nc.vector.reciprocal(rec, ndred[0:1, HALF:])
            tile.add_dep_helper(rec_i.ins, last_add.ins, sync=False,
                                reason="tail after stage-2 adds in DVE FIFO")
            vw_bf = sb.tile([1, HALF], BF, tag="vwbf")
            nc.vector.tensor_mul(
                vw_bf.rearrange("p (g c) -> p g c", c=4),
                ndred[0:1, 0:HALF].rearrange("p (g c) -> p g c", c=4),
                rec[0:1, :, None].to_broadcast([1, GRP, 4]))
            vwc = sb.tile([P, KC], BF, tag="vwc")
            for k in range(KC):
                pt = ps.tile([P, 1], F32, tag="pA", bufs=4, name=f"pvw{k}")
                mm = nc.tensor.matmul(pt, lhsT=vw_bf[0:1, k * P:(k + 1) * P],
                                      rhs=ones_bf[0:1, 0:1], start=True, stop=True)
                if k == 0:
                    tile.add_dep_helper(mm.ins, last_mm.ins, sync=False,
                                        reason="tail after stage-2 in PE FIFO")
                nc.scalar.copy(vwc[:, k:k + 1], pt)
            orow = sb.tile([1, HALF], F32, tag="orow")
            for h in range(2):
                pm = ps.tile([1, 512], F32, tag="pB", bufs=4, name=f"po{h}")
                for k in range(KC):
                    nc.tensor.matmul(pm, lhsT=vwc[:, k:k + 1],
                                     rhs=wo_t[k][:, h * 512:(h + 1) * 512],
                                     start=(k == 0),
                                     stop=(not with_bias and k == KC - 1))
                if with_bias:
                    nc.tensor.matmul(pm, lhsT=ones_bf[0:1, 0:1],
                                     rhs=bor[0:1, h * 512:(h + 1) * 512],
                                     start=False, stop=True)
                nc.scalar.copy(orow[0:1, h * 512:(h + 1) * 512], pm)
            # broadcast out_row to 128 partitions, then DMA-accumulate it onto
            # the pre-copied x1 base (CCE add in the DMA datapath).
            obc = sb.tile([P, HALF], F32, tag="obc", name="obc")
            for h in range(2):
                pb = ps.tile([P, 512], F32, tag="pB", bufs=4, name=f"pbc{h}")
                nc.tensor.matmul(pb, lhsT=ones_f[0:1, :],
                                 rhs=orow[0:1, h * 512:(h + 1) * 512],
                                 start=True, stop=True)
                nc.scalar.copy(obc[:, h * 512:(h + 1) * 512], pb)
            acc = nc.gpsimd.dma_start(
                out=out[:, 0:HALF].rearrange("(a p) d -> p a d", p=P),
                in_=obc[:, None, :].to_broadcast([P, TC, HALF]),
                accum_op=ALU.add)
            tile.add_dep_helper(acc.ins, d2d.ins, sync=True,
                                reason="y1 accumulate after DRAM base copy")

    nc.compile()
    return nc


def _get_program(with_bias: bool):
    key = ("nc", with_bias)
    if key not in _CACHE:
        _CACHE[key] = _build_program(with_bias)
    return _CACHE[key]


def kernel(**inputs) -> np.ndarray:
    x = np.asarray(inputs["x"], np.float32)
    n1_g = np.asarray(inputs["n1_g"], np.float32)
    n1_b = np.asarray(inputs["n1_b"], np.float32)
    wq = np.asarray(inputs["wq"], np.float32)
    bq = np.asarray(inputs["bq"], np.float32)
    wk = np.asarray(inputs["wk"], np.float32)
    bk = np.asarray(inputs["bk"], np.float32)
    wv = np.asarray(inputs["wv"], np.float32)
    bv = np.asarray(inputs["bv"], np.float32)
    wo = np.asarray(inputs["wo"], np.float32)
    bo = np.asarray(inputs["bo"], np.float32)
    n2_g = np.asarray(inputs["n2_g"], np.float32)
    n2_b = np.asarray(inputs["n2_b"], np.float32)
    f1 = np.asarray(inputs["f1"], np.float32)
    b1 = np.asarray(inputs["b1"], np.float32)
    f2 = np.asarray(inputs["f2"], np.float32)
    b2 = np.asarray(inputs["b2"], np.float32)

    isr = 1.0 / math.sqrt(RANK)
    # fold LN affine: gamma into weight rows, beta into effective bias rows
    F1s = f1.sum(0)
    F2s = f2.sum(0)
    W1 = (n2_g[:, None] * F1s) * isr
    b1e = (n2_b @ F1s) * isr + b1
    Wqkv = np.concatenate([n1_g[:, None] * wq.T, n1_g[:, None] * wk.T,
                           n1_g[:, None] * wv.T], axis=1)
    bqkve = np.concatenate([n1_b @ wq.T + bq, n1_b @ wk.T + bk,
                            n1_b @ wv.T + bv])

    with_bias = bool(np.any(bqkve) or np.any(b2) or np.any(bo))

    wqkv_bf = Wqkv.astype(BF16)
    f1_bf = W1.astype(BF16)
    f2_bf = (F2s * isr).astype(BF16)
    woT_bf = np.ascontiguousarray(wo.T).astype(BF16)

    xf = np.ascontiguousarray(x.reshape(B * T, DIM))
    shared = {
        "wqkv": wqkv_bf,
        "f1w": f1_bf,
        "f2w": f2_bf,
        "woT": woT_bf,
        "b1e": np.ascontiguousarray(b1e.reshape(HALF, 1), dtype=np.float32),
    }
    if with_bias:
        shared["bqkve"] = np.ascontiguousarray(bqkve.reshape(1, -1)).astype(BF16)
        shared["b2e"] = np.ascontiguousarray(b2.reshape(1, -1)).astype(BF16)
        shared["boe"] = np.ascontiguousarray(bo.reshape(1, -1)).astype(BF16)
    in_maps = []
    for i in range(NCORES):
        rows = xf[i * ROWS:(i + 1) * ROWS]
        m = dict(shared)
        m["xc"] = rows
        m["xT"] = rows.T.astype(BF16, order="C")
        in_maps.append(m)

    nc = _get_program(with_bias)
    res = run_bass_kernel_spmd(nc, in_maps, core_ids=list(range(NCORES)))
    global _LAST_RESULTS
    _LAST_RESULTS = res
    y = np.concatenate([res.results[i]["out"] for i in range(NCORES)], axis=0)
    return np.ascontiguousarray(y.reshape(B, T, DIM))
